# revision 47
# baseline (speedup 1.0000x reference)
"""Trainium2 Bass kernel for a GPT-2 style transformer block (post-LN).

Reference computation (B=4, S=2048, D=1024, H=16, dh=64, F=4096, fp32):
    qkv = x @ Wqkv + bqkv ; causal MHA ; attn_out = ctx @ Wo + bo
    h = LN(attn_out + x; g1, b1)
    m = gelu_exact(h @ Wfc + bfc) @ Wp + bp
    out = LN(m + h; g2, b2)

Sharding (8 cores, no collectives): core c = 2*b + p owns batch b and an
interleaved set of eight 128-row query tiles G(p) chosen so both cores of a
batch pair have identical causal work per local tile index j:
    G(0) = [0,3,4,7,8,11,12,15],  G(1) = [1,2,5,6,9,10,13,14]
At local q-tile j each core processes k-tiles 0..2j+1 (uniform trip counts
across cores); the two boundary k-tiles {2j, 2j+1} are masked with a
per-core additive maskT passed as data. Matmuls run in bf16 with fp32 PSUM
accumulation; softmax runs without max-subtraction (scores are O(1) for this
problem's data) and the denominator comes from a ones-column appended to V.

Attention works in "scoresT" layout [k, q] so the probabilities feed the
attn@V matmul directly as the moving operand (no per-tile transposes of the
probability matrix); the per-query normalization happens on the much smaller
ctx tensor after a [65,128] PE transpose brings it token-major.
"""

import numpy as np
import ml_dtypes

import concourse.bass as bass
import concourse.bacc as bacc
import concourse.mybir as mybir
import concourse.tile as tile
from concourse import bass_utils
from concourse.masks import make_identity

BF16 = mybir.dt.bfloat16
F32 = mybir.dt.float32
AF = mybir.ActivationFunctionType
ADD = mybir.AluOpType.add
MULT = mybir.AluOpType.mult

D, S, H, dh, F = 1024, 2048, 16, 64, 4096
R = 1024                # q rows per core
NT = S // 128           # 16 k-tiles
JT = R // 128           # 8 local q-tiles
DC = D // 128           # 8 contraction chunks of D
FG = 4                  # MLP hidden stream groups (1024 each)
EPS = 1e-5
NEG = -1e9

G_EVEN = [0, 3, 4, 7, 8, 11, 12, 15]
G_ODD = [1, 2, 5, 6, 9, 10, 13, 14]

nbf16 = ml_dtypes.bfloat16


def build_nc():
    nc = bacc.Bacc("TRN2", target_bir_lowering=False, debug=False, num_devices=8)

    xT = nc.dram_tensor("xT", [D, S], BF16, kind="ExternalInput").ap()
    xqT = nc.dram_tensor("xqT", [D, R], BF16, kind="ExternalInput").ap()
    xres = nc.dram_tensor("xres", [R, D], F32, kind="ExternalInput").ap()
    maskT = nc.dram_tensor("maskT", [128, S], BF16, kind="ExternalInput").ap()
    wqkv = nc.dram_tensor("wqkv", [D, 3 * D], BF16, kind="ExternalInput").ap()
    bq_d = nc.dram_tensor("bq", [128, 8], F32, kind="ExternalInput").ap()
    bk_d = nc.dram_tensor("bk", [128, 8], F32, kind="ExternalInput").ap()
    bv_d = nc.dram_tensor("bv_b", [128, D], F32, kind="ExternalInput").ap()
    wo = nc.dram_tensor("wo", [D, D], BF16, kind="ExternalInput").ap()
    wfc = nc.dram_tensor("wfc", [D, F], BF16, kind="ExternalInput").ap()
    bfc_d = nc.dram_tensor("bfc_t", [128, 32], F32, kind="ExternalInput").ap()
    wp = nc.dram_tensor("wp", [F, D], BF16, kind="ExternalInput").ap()
    bp_d = nc.dram_tensor("bp_b", [128, D], F32, kind="ExternalInput").ap()
    g1_d = nc.dram_tensor("g1_b", [128, D], F32, kind="ExternalInput").ap()
    b1_d = nc.dram_tensor("b1_b", [128, D], F32, kind="ExternalInput").ap()
    g2_d = nc.dram_tensor("g2_b", [128, D], F32, kind="ExternalInput").ap()
    b2_d = nc.dram_tensor("b2_b", [128, D], F32, kind="ExternalInput").ap()
    out_d = nc.dram_tensor("out", [R, D], F32, kind="ExternalOutput").ap()

    with tile.TileContext(nc) as tc:
        with tc.tile_pool(name="const", bufs=1) as cpool:
            def load(name, dram, shape):
                t = cpool.tile(shape, F32, tag=name)
                nc.gpsimd.dma_start(t[:], dram)
                return t

            id32 = cpool.tile([128, 128], F32, tag="id32")
            make_identity(nc, id32[:])
            id16 = cpool.tile([128, 128], BF16, tag="id16")
            make_identity(nc, id16[:])
            mask_sb = cpool.tile([128, S], BF16, tag="mask")
            nc.gpsimd.dma_start(mask_sb[:], maskT)
            bq_sb = load("bq", bq_d, [128, 8])
            bk_sb = load("bk", bk_d, [128, 8])
            bv_sb = load("bv", bv_d, [128, D])
            bfc_sb = load("bfc", bfc_d, [128, 32])
            bp_sb = load("bp", bp_d, [128, D])
            g1_sb = load("g1", g1_d, [128, D])
            b1_sb = load("b1", b1_d, [128, D])
            g2_sb = load("g2", g2_d, [128, D])
            b2_sb = load("b2", b2_d, [128, D])
            eps_sb = cpool.tile([128, 1], F32, tag="eps")
            nc.vector.memset(eps_sb[:], EPS)

            _body(nc, tc, xT, xqT, xres, wqkv, wo, wfc, wp, out_d,
                  id32, id16, mask_sb, bq_sb, bk_sb, bv_sb,
                  bfc_sb, bp_sb, g1_sb, b1_sb, g2_sb, b2_sb, eps_sb)

    nc.compile()
    return nc


def _body(nc, tc, xT, xqT, xres, wqkv, wo, wfc, wp, out_d,
          id32, id16, mask_sb, bq_sb, bk_sb, bv_sb,
          bfc_sb, bp_sb, g1_sb, b1_sb, g2_sb, b2_sb, eps_sb):
    from contextlib import ExitStack
    _ctx_stack = ExitStack()
    if True:
      with tc.tile_pool(name="qkvp", bufs=1) as qkvp:
        q_sb = qkvp.tile([128, 8, R], BF16, tag="q")       # [2*dh, hpair, tok]
        k_sb = qkvp.tile([128, 8, S], BF16, tag="k")
        v_sb = qkvp.tile([128, NT, H, dh + 1], BF16, tag="v")  # +ones col

        # ---------------- phase A: QKV projections ------------------------
        with tc.tile_pool(name="xt", bufs=1) as xtp:
            xt_sb = xtp.tile([128, DC, S], BF16, tag="xt")

            with (tc.tile_pool(name="xq", bufs=1) as xqp,
                  tc.tile_pool(name="wq", bufs=1) as wqp,
                  tc.tile_pool(name="psA", bufs=2, space="PSUM") as psA):
                xq_sb = xqp.tile([128, DC, R], BF16, tag="xq")
                for c in range(DC):
                    nc.sync.dma_start(
                        xq_sb[:, c, :], xqT[128 * c:128 * (c + 1), :])
                wq_sb = wqp.tile([128, DC, D], BF16, tag="wq")
                for c in range(DC):
                    nc.sync.dma_start(
                        wq_sb[:, c, :], wqkv[128 * c:128 * (c + 1), 0:D])
                for c in range(DC):
                    nc.sync.dma_start(
                        xt_sb[:, c, :], xT[128 * c:128 * (c + 1), :])
                for t in range(8):
                    ps = psA.tile([128, R], F32, tag="psq")
                    for d in range(DC):
                        for tb in range(2):
                            nc.tensor.matmul(
                                ps[:, 512 * tb:512 * (tb + 1)],
                                wq_sb[:, d, 128 * t:128 * (t + 1)],
                                xq_sb[:, d, 512 * tb:512 * (tb + 1)],
                                start=(d == 0), stop=(d == DC - 1))
                    nc.scalar.activation(
                        q_sb[:, t, :], ps[:],
                        AF.Identity, bias=bq_sb[:, t:t + 1])

            with (tc.tile_pool(name="wkv", bufs=2) as wkvp,
                  tc.tile_pool(name="psA2", bufs=2, space="PSUM") as psA2):
                wk_sb = wkvp.tile([128, DC, D], BF16, tag="wkv")
                nc.sync.dma_start(
                    wk_sb[:],
                    wqkv[:, D:2 * D].rearrange("(c p) n -> p c n", p=128))
                for t in range(8):
                    for half in range(2):
                        ps = psA2.tile([128, R], F32, tag="psk")
                        for d in range(DC):
                            for tb in range(2):
                                nc.tensor.matmul(
                                    ps[:, 512 * tb:512 * (tb + 1)],
                                    wk_sb[:, d, 128 * t:128 * (t + 1)],
                                    xt_sb[:, d, 1024 * half + 512 * tb:
                                          1024 * half + 512 * (tb + 1)],
                                    start=(d == 0), stop=(d == DC - 1))
                        nc.scalar.activation(
                            k_sb[:, t, 1024 * half:1024 * (half + 1)],
                            ps[:], AF.Identity, bias=bk_sb[:, t:t + 1])

                wv_sb = wkvp.tile([128, DC, D], BF16, tag="wkv")
                nc.sync.dma_start(
                    wv_sb[:],
                    wqkv[:, 2 * D:3 * D].rearrange("(c p) n -> p c n", p=128))
                nc.vector.memset(v_sb[:, :, :, dh:dh + 1], 1.0)
                for ki in range(NT):
                    ps = psA2.tile([128, R], F32, tag="psv")
                    for d in range(DC):
                        for hf in range(2):
                            nc.tensor.matmul(
                                ps[:, 512 * hf:512 * (hf + 1)],
                                xt_sb[:, d, 128 * ki:128 * (ki + 1)],
                                wv_sb[:, d, 512 * hf:512 * (hf + 1)],
                                start=(d == 0), stop=(d == DC - 1))
                    nc.vector.tensor_tensor(ps[:], ps[:], bv_sb[:], ADD)
                    nc.scalar.copy(
                        v_sb[:, ki, :, 0:dh],
                        ps[:].rearrange("p (h d) -> p h d", d=dh))

        # ---------------- phase B: attention ------------------------------
        ctxp = _ctx_stack.enter_context(
            tc.tile_pool(name="ctxp", bufs=1, side="right"))
        ctxT_sb = ctxp.tile([128, DC, R], BF16, tag="ctxT")
        with (tc.tile_pool(name="probs", bufs=3) as prp,
              tc.tile_pool(name="psS", bufs=3, space="PSUM") as psS,
              tc.tile_pool(name="psC", bufs=2, space="PSUM") as psC,
              tc.tile_pool(name="cta", bufs=2) as ctap,
              tc.tile_pool(name="rtile", bufs=4) as rpool):
            for h in range(H):
                po = 64 * (h % 2)
                hp = h // 2
                for Q in range(2):
                    w0 = 512 * Q
                    ctx_ps = psC.tile([dh + 1, 512], F32, tag="ctxaug")
                    for m2 in range(4 * (Q + 1)):
                        wstart = max(w0, 128 * m2)
                        qn = w0 + 512 - wstart
                        sc = psS.tile([128, 2, 512], F32, tag="sc")
                        for kk in range(2):
                            ki = 2 * m2 + kk
                            nc.tensor.matmul(
                                sc[:, kk, 0:qn],
                                k_sb[po:po + 64, hp, 128 * ki:128 * (ki + 1)],
                                q_sb[po:po + 64, hp, wstart:wstart + qn],
                                start=True, stop=True)
                        if Q == m2 // 4:
                            nc.vector.tensor_tensor(
                                sc[:, :, 0:128], sc[:, :, 0:128],
                                mask_sb[:, 256 * m2:256 * (m2 + 1)].rearrange(
                                    "p (k c) -> p k c", k=2), ADD)
                        pr = prp.tile([128, 2, 512], BF16, tag="pr")
                        nc.scalar.activation(
                            pr[:, :, 0:qn], sc[:, :, 0:qn], AF.Exp, scale=0.125)
                        for kk in range(2):
                            ki = 2 * m2 + kk
                            nc.tensor.matmul(
                                ctx_ps[:, wstart - w0:wstart - w0 + qn],
                                v_sb[:, ki, h, :],
                                pr[:, kk, 0:qn],
                                start=(m2 == 0 and kk == 0),
                                stop=(m2 == 4 * Q + 3 and kk == 1),
                                skip_group_check=True)
                    cta_sb = ctap.tile([dh + 1, 512], F32, tag="cta")
                    nc.scalar.copy(cta_sb[:], ctx_ps[:])
                    rden = rpool.tile([1, 512], F32, tag="r")
                    nc.vector.reciprocal(rden[:], cta_sb[dh:dh + 1, :])
                    rb = rpool.tile([dh, 512], F32, tag="rb")
                    nc.gpsimd.partition_broadcast(rb[:], rden[:], channels=dh)
                    nc.vector.tensor_tensor(
                        ctxT_sb[po:po + dh, hp, 512 * Q:512 * (Q + 1)],
                        cta_sb[0:dh, :], rb[:], MULT)


      # ------------------ phase C: out-proj + residual + LN1 --------------
      with tc.tile_pool(name="acts", bufs=1) as apool:
        h_sb = apool.tile([128, JT, D], F32, tag="h")
        with (tc.tile_pool(name="wo", bufs=1) as wop,
              tc.tile_pool(name="xres", bufs=1) as xrp,
              tc.tile_pool(name="psao", bufs=2, space="PSUM") as psaop,
              tc.tile_pool(name="stats", bufs=4) as stp):
            wo_sb = wop.tile([128, DC, D], BF16, tag="wo")
            nc.sync.dma_start(wo_sb[:], wo.rearrange("(c p) n -> p c n", p=128))
            xres_sb = xrp.tile([128, JT, D], F32, tag="xres")
            nc.sync.dma_start(
                xres_sb[:], xres.rearrange("(j p) d -> p j d", p=128))

            # out-proj directly token-major: ctxT chunks stationary, Wo moving.
            # bo is pre-added into xres on the host.
            for j in range(JT):
                ps = psaop.tile([128, D], F32, tag="psao")
                for c in range(DC):
                    for ob in range(2):
                        nc.tensor.matmul(
                            ps[:, 512 * ob:512 * (ob + 1)],
                            ctxT_sb[:, c, 128 * j:128 * (j + 1)],
                            wo_sb[:, c, 512 * ob:512 * (ob + 1)],
                            start=(c == 0), stop=(c == DC - 1))
                nc.vector.tensor_tensor(
                    h_sb[:, j, :], ps[:], xres_sb[:, j, :], ADD)
                _layernorm(nc, stp, h_sb, j, g1_sb, b1_sb, eps_sb)

        _ctx_stack.close()  # frees ctx tiles before MLP
        # ---------------- phase D: MLP + LN2 ------------------------------
        with (tc.tile_pool(name="hT", bufs=1) as htp,
              tc.tile_pool(name="wfc", bufs=2) as wfp,
              tc.tile_pool(name="wp", bufs=2) as wpp,
              tc.tile_pool(name="aT", bufs=1) as atp,
              tc.tile_pool(name="m", bufs=1) as mp,
              tc.tile_pool(name="tph", bufs=2, space="PSUM") as tphp,
              tc.tile_pool(name="psfc", bufs=2, space="PSUM") as psfcp,
              tc.tile_pool(name="psm", bufs=2, space="PSUM") as psmp,
              tc.tile_pool(name="stats2", bufs=4) as stp2):
            hT_sb = htp.tile([128, DC, R], BF16, tag="hT")
            for j in range(JT):
                for c in range(DC):
                    tp = tphp.tile([128, 128], F32, tag="tph")
                    nc.tensor.transpose(
                        tp[:], h_sb[:, j, 128 * c:128 * (c + 1)], id32[:])
                    nc.vector.tensor_copy(hT_sb[:, c, 128 * j:128 * (j + 1)], tp[:])

            m_sb = mp.tile([128, JT, D], F32, tag="m")
            for j in range(JT):
                nc.gpsimd.tensor_tensor(m_sb[:, j, :], h_sb[:, j, :],
                                        bp_sb[:], ADD)
            for fg in range(FG):
                wfc_sb = wfp.tile([128, DC, 1024], BF16, tag="wfc")
                nc.sync.dma_start(
                    wfc_sb[:],
                    wfc[:, 1024 * fg:1024 * (fg + 1)].rearrange(
                        "(c p) n -> p c n", p=128))
                aT_sb = atp.tile([128, 8, R], BF16, tag="aT")
                for hi in range(8):
                    for qb in range(2):
                        ps = psfcp.tile([128, 512], F32, tag="psfc")
                        for d in range(DC):
                            nc.tensor.matmul(
                                ps[:],
                                wfc_sb[:, d, 128 * hi:128 * (hi + 1)],
                                hT_sb[:, d, 512 * qb:512 * (qb + 1)],
                                start=(d == 0), stop=(d == DC - 1))
                        nc.scalar.activation(
                            aT_sb[:, hi, 512 * qb:512 * (qb + 1)], ps[:],
                            AF.Gelu,
                            bias=bfc_sb[:, 8 * fg + hi:8 * fg + hi + 1])
                wp_sb = wpp.tile([128, 8, D], BF16, tag="wp")
                nc.sync.dma_start(
                    wp_sb[:],
                    wp[1024 * fg:1024 * (fg + 1), :].rearrange(
                        "(c p) n -> p c n", p=128))
                for j in range(JT):
                    ps = psmp.tile([128, D], F32, tag="psm")
                    for hc in range(8):
                        for ob in range(2):
                            nc.tensor.matmul(
                                ps[:, 512 * ob:512 * (ob + 1)],
                                aT_sb[:, hc, 128 * j:128 * (j + 1)],
                                wp_sb[:, hc, 512 * ob:512 * (ob + 1)],
                                start=(hc == 0), stop=(hc == 7))
                    nc.vector.tensor_tensor(
                        m_sb[:, j, :], m_sb[:, j, :], ps[:], ADD)

            for j in range(JT):
                _layernorm(nc, stp2, m_sb, j, g2_sb, b2_sb, eps_sb)
                nc.sync.dma_start(out_d[128 * j:128 * (j + 1), :], m_sb[:, j, :])


def _layernorm(nc, stp, buf, j, g_sb, b_sb, eps_sb, tail_eng=None):
    """LayerNorm over the free dim (D=1024) of buf[:, j, :] (fp32), in place."""
    st = stp.tile([128, 12], F32, tag="st")
    nc.vector.bn_stats(st[:, 0:6], buf[:, j, 0:512])
    nc.vector.bn_stats(st[:, 6:12], buf[:, j, 512:1024])
    mv = stp.tile([128, 2], F32, tag="mv")
    nc.vector.bn_aggr(mv[:], st[:])
    std = stp.tile([128, 1], F32, tag="std")
    nc.scalar.activation(std[:], mv[:, 1:2], AF.Sqrt, bias=eps_sb[:])
    rstd = stp.tile([128, 1], F32, tag="rstd")
    nc.vector.reciprocal(rstd[:], std[:])
    nmr = stp.tile([128, 1], F32, tag="nmr")
    nc.vector.tensor_scalar(nmr[:], mv[:, 0:1], rstd[:], -1.0, MULT, MULT)
    # (x - mu) * rstd == x*rstd + (-mu*rstd), fused into one ACT op
    nc.scalar.activation(buf[:, j, :], buf[:, j, :], AF.Identity,
                         bias=nmr[:], scale=rstd[:])
    nc.vector.tensor_tensor(buf[:, j, :], buf[:, j, :], g_sb[:], MULT)
    nc.vector.tensor_tensor(buf[:, j, :], buf[:, j, :], b_sb[:], ADD)


# --------------------------------------------------------------------------
# host side
# --------------------------------------------------------------------------
_NC_CACHE = None


def _get_nc():
    global _NC_CACHE
    if _NC_CACHE is None:
        _NC_CACHE = build_nc()
    return _NC_CACHE


def _core_rows(p):
    G = G_EVEN if p == 0 else G_ODD
    rows = np.concatenate([np.arange(128 * g, 128 * (g + 1)) for g in G])
    return rows, G


def _make_maskT(G):
    m = np.zeros((128, S), np.float32)
    kk = np.arange(128)[:, None]
    qq = np.arange(128)[None, :]
    for ki in range(NT):
        g = G[ki // 2]
        vis = (128 * ki + kk) <= (128 * g + qq)
        m[:, 128 * ki:128 * (ki + 1)] = np.where(vis, 0.0, NEG)
    return m


def kernel(x, mask, Wqkv, bqkv, Wo, bo, g1, b1, Wfc, bfc, Wp, bp, g2, b2):
    x = np.asarray(x, np.float32)
    Wqkv = np.asarray(Wqkv, np.float32)
    bqkv = np.asarray(bqkv, np.float32)
    Wo = np.asarray(Wo, np.float32)
    bo = np.asarray(bo, np.float32)
    Wfc = np.asarray(Wfc, np.float32)
    bfc = np.asarray(bfc, np.float32)
    Wp = np.asarray(Wp, np.float32)
    bp = np.asarray(bp, np.float32)
    g1 = np.asarray(g1, np.float32)
    b1 = np.asarray(b1, np.float32)
    g2 = np.asarray(g2, np.float32)
    b2 = np.asarray(b2, np.float32)

    nc = _get_nc()

    rep = lambda v: np.broadcast_to(v[None, :], (128, v.shape[0])).copy()
    common = dict(
        wqkv=Wqkv.astype(nbf16),
        bq=np.ascontiguousarray(bqkv[:D].reshape(8, 128).T),
        bk=np.ascontiguousarray(bqkv[D:2 * D].reshape(8, 128).T),
        bv_b=rep(bqkv[2 * D:]),
        wo=Wo.astype(nbf16),
        wfc=Wfc.astype(nbf16),
        bfc_t=np.ascontiguousarray(bfc.reshape(32, 128).T),
        wp=Wp.astype(nbf16),
        bp_b=rep(bp),
        g1_b=rep(g1), b1_b=rep(b1), g2_b=rep(g2), b2_b=rep(b2),
    )
    in_maps = []
    row_sets = []
    for c in range(8):
        b, p = c // 2, c % 2
        rows, G = _core_rows(p)
        row_sets.append((b, rows))
        m = dict(common)
        m["xT"] = np.ascontiguousarray(x[b].T).astype(nbf16)
        m["xqT"] = np.ascontiguousarray(x[b][rows].T).astype(nbf16)
        m["xres"] = np.ascontiguousarray(x[b][rows]) + bo[None, :]
        m["maskT"] = _make_maskT(G).astype(nbf16)
        in_maps.append(m)

    res = bass_utils.run_bass_kernel_spmd(nc, in_maps, core_ids=list(range(8)))
    out = np.zeros((4, S, D), np.float32)
    for c in range(8):
        b, rows = row_sets[c]
        out[b][rows] = res.results[c]["out"]
    return out
